# revision 5
# baseline (speedup 1.0000x reference)
"""Trainium2 Bass kernel for nn_PolyEpisodicRNNModel.

Model (per reference):
    h0 = 0.001;  scan over S steps:
        z = clip(tanh(h) @ w_hh0.T + x_t @ w_ih + b1, 0, 1)
        h = z**poly_power @ w_hh1.T + b2          (hs collects h)
    readout per step: out = relu(hs @ hl1_w.T + hl1_b) @ ro_w.T + ro_b

Strategy:
  - Data-parallel over batch: B=128 -> 16 per core on 8 NeuronCores.
  - The scan is serial; per step the PE must stream both HxH weight
    matrices.  We keep the *weights* as the moving operand (N=512 per
    matmul, float32r -> 1 cycle/row) and the small [128,16] activation
    tiles as the stationary operand (cheap weight loads).
  - State is carried transposed ([H, Bshard], H on partitions) via PE
    transposes of the [16, 1024] z/h tiles each step.
  - hs is staged to DRAM transposed; a static phase-2 readout
    (W-stationary, n=S*16 moving) computes the output per core.
  - All layout transforms (weight transposes, x transpose) happen on the
    host in numpy: they are free w.r.t. HW time.
"""

import sys

for _p in ("/opt/trn_rl_repo",):
    if _p not in sys.path:
        sys.path.insert(0, _p)

import numpy as np

import concourse.bass as bass
import concourse.bacc as bacc
import concourse.mybir as mybir
import concourse.tile as tile
from concourse.bass import ds
from concourse.bass_utils import run_bass_kernel_spmd
from concourse.masks import make_identity

S, B, D, H, O = 512, 128, 256, 1024, 256
NCORES = 8
BS = B // NCORES            # 16 batch rows per core
KT = H // 128               # 8 contraction tiles over H
DTILES = D // 128           # 2 contraction tiles over D
OT = O // 128               # 2 output tiles
U = 16                      # scan steps unrolled per For_i iteration
N_TOT = S * BS              # 8192 readout columns per core
CHUNK = 512                 # readout moving-dim chunk
NCH = N_TOT // CHUNK        # 16 chunks

F32 = mybir.dt.float32
F32R = mybir.dt.float32r
AF = mybir.ActivationFunctionType
ALU = mybir.AluOpType

TANH_H0 = float(np.tanh(0.001))

_BUILD_CACHE = {}
LAST_RESULTS = None


def _r(ap):
    """View an fp32 AP as float32r for full-speed PE matmuls."""
    return ap.bitcast(F32R)


def _build(poly_power: int):
    nc = bacc.Bacc("TRN2", target_bir_lowering=False, debug=False,
                   num_devices=NCORES)

    # ---- DRAM parameters (host-prepped layouts) ----
    xT_d = nc.dram_tensor("xT", [DTILES, 128, S, BS], F32, kind="ExternalInput")
    w0T_d = nc.dram_tensor("w0T", [128, KT * H], F32, kind="ExternalInput")
    wih_d = nc.dram_tensor("wih", [128, DTILES * H], F32, kind="ExternalInput")
    w1T_d = nc.dram_tensor("w1T", [128, KT * H], F32, kind="ExternalInput")
    hl1T_d = nc.dram_tensor("hl1T", [128, KT * H], F32, kind="ExternalInput")
    roT_d = nc.dram_tensor("roT", [128, KT * O], F32, kind="ExternalInput")
    b1bc_d = nc.dram_tensor("b1bc", [BS, H], F32, kind="ExternalInput")
    b2bc_d = nc.dram_tensor("b2bc", [BS, H], F32, kind="ExternalInput")
    hl1bc_d = nc.dram_tensor("hl1bc", [128, KT], F32, kind="ExternalInput")
    robc_d = nc.dram_tensor("robc", [128, OT], F32, kind="ExternalInput")

    outT_d = nc.dram_tensor("outT", [OT, 128, N_TOT], F32, kind="ExternalOutput")
    hsT_d = nc.dram_tensor("hsT", [KT, 128, N_TOT], F32R)  # internal scratch

    with tile.TileContext(nc) as tc:
        with tc.tile_pool(name="const", bufs=1) as cpool:
            idn = cpool.tile([128, 128], F32, tag="idn")
            make_identity(nc, idn[:])
            idn16 = idn[0:BS, 0:BS]

            # loop-carried state: tanh(h).T  [128, KT*BS]
            tanhT_state = cpool.tile([128, KT * BS], F32R, tag="tanhT")

            # ---- phase 1: the scan ----
            with (
                tc.tile_pool(name="sw", bufs=1) as swpool,
                tc.tile_pool(name="work", bufs=1) as wpool,
                tc.tile_pool(name="xin", bufs=2) as xpool,
                tc.tile_pool(name="hout", bufs=1) as hpool,
                tc.tile_pool(name="ps1", bufs=1, space="PSUM") as pp1,
                tc.tile_pool(name="ps2", bufs=2, space="PSUM") as pp2,
            ):
                w0T = swpool.tile([128, KT * H], F32R, tag="w0T")
                wih = swpool.tile([128, DTILES * H], F32R, tag="wih")
                w1T = swpool.tile([128, KT * H], F32R, tag="w1T")
                b1bc = swpool.tile([BS, H], F32, tag="b1bc")
                b2bc = swpool.tile([BS, H], F32, tag="b2bc")
                for t_, d_ in ((b1bc, b1bc_d), (b2bc, b2bc_d)):
                    nc.sync.dma_start(out=t_[:], in_=d_[:])
                with tc.tile_pool(name="stg", bufs=2) as stgpool:
                    for t_, d_ in ((w0T, w0T_d), (wih, wih_d), (w1T, w1T_d)):
                        stg = stgpool.tile(list(t_.shape), F32, tag="stg")
                        nc.sync.dma_start(out=stg[:], in_=d_[:])
                        nc.vector.tensor_copy(t_[:], stg[:])
                # init state: tanh(h0) converted to f32r
                stg0 = swpool.tile([128, KT * BS], F32, tag="stg0")
                nc.vector.memset(stg0[:], TANH_H0)
                nc.vector.tensor_copy(tanhT_state[:], stg0[:])

                with tc.For_i(0, S, U) as t0:
                    # batched x loads for this body: [128, 2, U*BS]
                    xstg = xpool.tile([128, DTILES, U * BS], F32, tag="xstg")
                    for dt_ in range(DTILES):
                        nc.sync.dma_start(out=xstg[:, dt_, :],
                                          in_=xT_d[dt_, :, ds(t0, U), :])
                    xblk = xpool.tile([128, DTILES, U * BS], F32R, tag="xb")
                    nc.vector.tensor_copy(xblk[:], xstg[:])
                    # h outputs accumulated for this body
                    hTblk = hpool.tile([128, U, KT * BS], F32R, tag="hTblk")

                    cur = tanhT_state
                    for u in range(U):
                        # -- GEMM1: z[b, j] = tanh(h) @ w0.T + x_t @ w_ih --
                        pz = pp1.tile([BS, H], F32, tag="pz")
                        for half in range(2):
                            o0 = half * 512
                            for kt in range(KT):
                                nc.tensor.matmul(
                                    pz[:, o0:o0 + 512],
                                    cur[:, kt * BS:(kt + 1) * BS],
                                    w0T[:, kt * H + o0: kt * H + o0 + 512],
                                    start=(kt == 0), stop=False)
                            for dt_ in range(DTILES):
                                nc.tensor.matmul(
                                    pz[:, o0:o0 + 512],
                                    xblk[:, dt_, u * BS:(u + 1) * BS],
                                    wih[:, dt_ * H + o0: dt_ * H + o0 + 512],
                                    start=False, stop=(dt_ == DTILES - 1))
                        # z = clip(pz + b1, 0, 1)
                        znt = wpool.tile([BS, H], F32, tag="znt")
                        nc.vector.tensor_add(znt[:], pz[:], b1bc[:])
                        nc.vector.tensor_scalar(znt[:], znt[:], 0.0, 1.0,
                                                op0=ALU.max, op1=ALU.min)
                        # transpose z -> [128, KT*BS] (PE, 16-col blocks)
                        pzT = pp2.tile([128, KT * BS], F32, tag="pzT")
                        for kt in range(KT):
                            nc.tensor.transpose(
                                pzT[:, kt * BS:(kt + 1) * BS],
                                znt[0:BS, kt * 128:(kt + 1) * 128], idn16)
                        # z2T = zT ** poly_power
                        z2T = wpool.tile([128, KT * BS], F32R, tag="z2T")
                        if poly_power == 2:
                            nc.scalar.activation(z2T[:], pzT[:], AF.Square)
                        elif poly_power == 1:
                            nc.scalar.activation(z2T[:], pzT[:], AF.Copy)
                        elif poly_power == 0:
                            nc.vector.memset(z2T[:], 1.0)
                        else:
                            zT = wpool.tile([128, KT * BS], F32, tag="zTg")
                            nc.scalar.activation(zT[:], pzT[:], AF.Copy)
                            zp = wpool.tile([128, KT * BS], F32, tag="zpg")
                            nc.scalar.activation(zp[:], pzT[:], AF.Square)
                            for _ in range(poly_power - 2):
                                nc.vector.tensor_mul(zp[:], zp[:], zT[:])
                            nc.vector.tensor_copy(z2T[:], zp[:])
                        # -- GEMM2: h[b, j'] = z2 @ w1.T --
                        ph = pp1.tile([BS, H], F32, tag="ph")
                        for half in range(2):
                            o0 = half * 512
                            for kt in range(KT):
                                nc.tensor.matmul(
                                    ph[:, o0:o0 + 512],
                                    z2T[:, kt * BS:(kt + 1) * BS],
                                    w1T[:, kt * H + o0: kt * H + o0 + 512],
                                    start=(kt == 0), stop=(kt == KT - 1))
                        hnt = wpool.tile([BS, H], F32, tag="hnt")
                        nc.vector.tensor_add(hnt[:], ph[:], b2bc[:])
                        # transpose h -> [128, KT*BS]
                        phT = pp2.tile([128, KT * BS], F32, tag="phT")
                        for kt in range(KT):
                            nc.tensor.transpose(
                                phT[:, kt * BS:(kt + 1) * BS],
                                hnt[0:BS, kt * 128:(kt + 1) * 128], idn16)
                        # stage hT for DRAM; tanh for next step
                        nc.scalar.activation(hTblk[:, u, :], phT[:], AF.Copy)
                        nxt = tanhT_state if u == U - 1 else \
                            wpool.tile([128, KT * BS], F32R, tag="thnx")
                        nc.scalar.activation(nxt[:], phT[:], AF.Tanh)
                        cur = nxt

                    for kt in range(KT):
                        nc.sync.dma_start(
                            out=hsT_d[kt, :, ds(t0 * BS, U * BS)],
                            in_=hTblk[:, :, kt * BS:(kt + 1) * BS])

            # ---- phase 2: readout ----
            with (
                tc.tile_pool(name="rw", bufs=1) as rwpool,
                tc.tile_pool(name="p2", bufs=1) as p2pool,
                tc.tile_pool(name="p2s", bufs=2) as p2spool,
                tc.tile_pool(name="ps3", bufs=2, space="PSUM") as pp3,
            ):
                hl1T = rwpool.tile([128, KT * H], F32R, tag="hl1T")
                roT = rwpool.tile([128, KT * O], F32R, tag="roT")
                hl1bc = rwpool.tile([128, KT], F32, tag="hl1bc")
                robc = rwpool.tile([128, OT], F32, tag="robc")
                for t_, d_ in ((hl1bc, hl1bc_d), (robc, robc_d)):
                    nc.sync.dma_start(out=t_[:], in_=d_[:])
                with tc.tile_pool(name="stg2", bufs=2) as stg2pool:
                    for t_, d_ in ((hl1T, hl1T_d), (roT, roT_d)):
                        stg = stg2pool.tile(list(t_.shape), F32, tag="stg2")
                        nc.sync.dma_start(out=stg[:], in_=d_[:])
                        nc.vector.tensor_copy(t_[:], stg[:])

                for c in range(NCH):
                    n0 = c * CHUNK
                    hsc = p2pool.tile([128, KT, CHUNK], F32R, tag="hsc")
                    for kt in range(KT):
                        nc.sync.dma_start(out=hsc[:, kt, :],
                                          in_=hsT_d[kt, :, n0:n0 + CHUNK])
                    hidT = p2pool.tile([128, KT, CHUNK], F32R, tag="hidT")
                    for gt in range(KT):
                        phid = pp3.tile([128, CHUNK], F32, tag="phid")
                        for ht in range(KT):
                            nc.tensor.matmul(
                                phid[:],
                                hl1T[:, ht * H + gt * 128:
                                     ht * H + (gt + 1) * 128],
                                hsc[:, ht, :],
                                start=(ht == 0), stop=(ht == KT - 1))
                        nc.scalar.activation(hidT[:, gt, :], phid[:], AF.Relu,
                                             bias=hl1bc[:, gt:gt + 1])
                    for ot in range(OT):
                        po = pp3.tile([128, CHUNK], F32, tag="po")
                        for gt in range(KT):
                            nc.tensor.matmul(
                                po[:],
                                roT[:, gt * O + ot * 128:
                                    gt * O + (ot + 1) * 128],
                                hidT[:, gt, :],
                                start=(gt == 0), stop=(gt == KT - 1))
                        osb = p2spool.tile([128, CHUNK], F32, tag="osb")
                        nc.vector.tensor_scalar_add(osb[:], po[:],
                                                    robc[:, ot:ot + 1])
                        nc.sync.dma_start(out=outT_d[ot, :, n0:n0 + CHUNK],
                                          in_=osb[:])

    nc.compile()
    return nc


def _get_nc(poly_power: int):
    if poly_power not in _BUILD_CACHE:
        _BUILD_CACHE[poly_power] = _build(poly_power)
    return _BUILD_CACHE[poly_power]


def _tile_kmajor(w, kt, width):
    """[K, width] -> [128, kt*width] with contraction tiled on partitions."""
    return np.ascontiguousarray(
        w.reshape(kt, 128, width).transpose(1, 0, 2).reshape(128, kt * width))


def kernel(x, w_ih, w_hh0, w_hh1, b1, b2, hl1_w, hl1_b, ro_w, ro_b,
           poly_power):
    global LAST_RESULTS
    x = np.asarray(x, np.float32)
    w_ih = np.asarray(w_ih, np.float32)
    w_hh0 = np.asarray(w_hh0, np.float32)
    w_hh1 = np.asarray(w_hh1, np.float32)
    b1 = np.asarray(b1, np.float32)
    b2 = np.asarray(b2, np.float32)
    hl1_w = np.asarray(hl1_w, np.float32)
    hl1_b = np.asarray(hl1_b, np.float32)
    ro_w = np.asarray(ro_w, np.float32)
    ro_b = np.asarray(ro_b, np.float32)
    p = int(poly_power)

    nc = _get_nc(p)

    # host layout prep (shared across cores)
    w0T = _tile_kmajor(np.ascontiguousarray(w_hh0.T), KT, H)
    wih = _tile_kmajor(w_ih, DTILES, H)
    w1T = _tile_kmajor(np.ascontiguousarray(w_hh1.T), KT, H)
    hl1T = _tile_kmajor(np.ascontiguousarray(hl1_w.T), KT, H)
    roT = _tile_kmajor(np.ascontiguousarray(ro_w.T), KT, O)
    b1bc = np.ascontiguousarray(np.broadcast_to(b1, (BS, H)))
    b2bc = np.ascontiguousarray(np.broadcast_to(b2, (BS, H)))
    hl1bc = np.ascontiguousarray(hl1_b.reshape(KT, 128).T)
    robc = np.ascontiguousarray(ro_b.reshape(OT, 128).T)

    # x: [S, B, D] -> per-core [DTILES, 128, S, BS]
    xT = np.ascontiguousarray(
        x.transpose(2, 0, 1).reshape(DTILES, 128, S, B))

    shared = dict(w0T=w0T, wih=wih, w1T=w1T, hl1T=hl1T, roT=roT,
                  b1bc=b1bc, b2bc=b2bc, hl1bc=hl1bc, robc=robc)
    in_maps = []
    for i in range(NCORES):
        m = dict(shared)
        m["xT"] = np.ascontiguousarray(xT[:, :, :, i * BS:(i + 1) * BS])
        in_maps.append(m)

    import os as _os
    _trace = _os.environ.get("KERNEL_TRACE", "") == "1"
    _kw = {}
    if _trace:
        _kw = dict(trace=True, tmpdir=_os.environ.get("KERNEL_TRACE_DIR") or None)
    res = run_bass_kernel_spmd(nc, in_maps, list(range(NCORES)), **_kw)
    LAST_RESULTS = res

    out = np.empty((S, B, O), np.float32)
    for i in range(NCORES):
        oT = res.results[i]["outT"]  # [OT, 128, S*BS]
        out[:, i * BS:(i + 1) * BS, :] = (
            oT.reshape(OT, 128, S, BS).transpose(2, 3, 0, 1)
            .reshape(S, BS, O))
    return out


# revision 11
# speedup vs baseline: 1.3441x; 1.3441x over previous
"""Trainium2 Bass kernel for nn_PolyEpisodicRNNModel.

Model (per reference):
    h0 = 0.001;  scan over S steps:
        z = clip(tanh(h) @ w_hh0.T + x_t @ w_ih + b1, 0, 1)
        h = z**poly_power @ w_hh1.T + b2          (hs collects h)
    readout per step: out = relu(hs @ hl1_w.T + hl1_b) @ ro_w.T + ro_b

Strategy:
  - Data-parallel over batch: B=128 -> 16 per core on 8 NeuronCores.
  - The scan is serial; per step the PE must stream both HxH weight
    matrices.  We keep the *weights* as the moving operand (N=512 per
    matmul, float32r -> 1 cycle/row) and the small [128,16] activation
    tiles as the stationary operand (cheap weight loads).
  - State is carried transposed ([H, Bshard], H on partitions) via PE
    transposes of the [16, 1024] z/h tiles each step.
  - hs is staged to DRAM transposed; a static phase-2 readout
    (W-stationary, n=S*16 moving) computes the output per core.
  - All layout transforms (weight transposes, x transpose) happen on the
    host in numpy: they are free w.r.t. HW time.
"""

import sys

for _p in ("/opt/trn_rl_repo",):
    if _p not in sys.path:
        sys.path.insert(0, _p)

import numpy as np

import concourse.bass as bass
import concourse.bacc as bacc
import concourse.mybir as mybir
import concourse.tile as tile
from concourse.bass import ds
from concourse.bass_utils import run_bass_kernel_spmd
from concourse.masks import make_identity

S, B, D, H, O = 512, 128, 256, 1024, 256
NCORES = 8
BS = B // NCORES            # 16 batch rows per core
KT = H // 128               # 8 contraction tiles over H
DTILES = D // 128           # 2 contraction tiles over D
OT = O // 128               # 2 output tiles
U = 16                      # scan steps unrolled per For_i iteration
HB = 4 * (B // NCORES)      # 64: half of the transposed state's columns
N_TOT = S * BS              # 8192 readout columns per core
CHUNK = 512                 # readout moving-dim chunk
NCH = N_TOT // CHUNK        # 16 chunks

F32 = mybir.dt.float32
F32R = mybir.dt.float32r
AF = mybir.ActivationFunctionType
ALU = mybir.AluOpType

TANH_H0 = float(np.tanh(0.001))

_BUILD_CACHE = {}
LAST_RESULTS = None


def _r(ap):
    """View an fp32 AP as float32r for full-speed PE matmuls."""
    return ap.bitcast(F32R)


def _build(poly_power: int):
    nc = bacc.Bacc("TRN2", target_bir_lowering=False, debug=False,
                   num_devices=NCORES)

    # ---- DRAM parameters (host-prepped layouts) ----
    xT_d = nc.dram_tensor("xT", [DTILES, 128, S, BS], F32, kind="ExternalInput")
    w0T_d = nc.dram_tensor("w0T", [128, KT * H], F32, kind="ExternalInput")
    wih_d = nc.dram_tensor("wih", [128, DTILES * H], F32, kind="ExternalInput")
    w1T_d = nc.dram_tensor("w1T", [128, KT * H], F32, kind="ExternalInput")
    hl1T_d = nc.dram_tensor("hl1T", [128, KT * H], F32, kind="ExternalInput")
    roT_d = nc.dram_tensor("roT", [128, KT * O], F32, kind="ExternalInput")
    b1bc_d = nc.dram_tensor("b1bc", [BS, H], F32, kind="ExternalInput")
    b2bc_d = nc.dram_tensor("b2bc", [BS, H], F32, kind="ExternalInput")
    hl1bc_d = nc.dram_tensor("hl1bc", [128, KT], F32, kind="ExternalInput")
    robc_d = nc.dram_tensor("robc", [128, OT], F32, kind="ExternalInput")

    outT_d = nc.dram_tensor("outT", [OT, 128, N_TOT], F32, kind="ExternalOutput")
    hsT_d = nc.dram_tensor("hsT", [KT, 128, N_TOT], F32R)  # internal scratch

    with tile.TileContext(nc) as tc:
        with tc.tile_pool(name="const", bufs=1) as cpool:
            idn = cpool.tile([128, 128], F32, tag="idn")
            make_identity(nc, idn[:])
            idn16 = idn[0:BS, 0:BS]

            # loop-carried state: tanh(h).T in two halves [128, 4*BS] each
            tanhT_state = [cpool.tile([128, HB], F32R, tag=f"tanhT{i}",
                                      name=f"tanhT{i}") for i in range(2)]

            # ---- phase 1: the scan ----
            with (
                tc.tile_pool(name="sw", bufs=1) as swpool,
                tc.tile_pool(name="work", bufs=1) as wpool,
                tc.tile_pool(name="xin", bufs=2) as xpool,
                tc.tile_pool(name="hout", bufs=1) as hpool,
                tc.tile_pool(name="ps1", bufs=1, space="PSUM") as pp1,
                tc.tile_pool(name="ps2", bufs=1, space="PSUM") as pp2,
            ):
                w0T = swpool.tile([128, KT * H], F32R, tag="w0T")
                wih = swpool.tile([128, DTILES * H], F32R, tag="wih")
                w1T = swpool.tile([128, KT * H], F32R, tag="w1T")
                b1bc = swpool.tile([BS, H], F32, tag="b1bc")
                b2bc = swpool.tile([BS, H], F32, tag="b2bc")
                for t_, d_ in ((b1bc, b1bc_d), (b2bc, b2bc_d)):
                    nc.sync.dma_start(out=t_[:], in_=d_[:])
                with tc.tile_pool(name="stg", bufs=2) as stgpool:
                    for t_, d_ in ((w0T, w0T_d), (wih, wih_d), (w1T, w1T_d)):
                        stg = stgpool.tile(list(t_.shape), F32, tag="stg")
                        nc.sync.dma_start(out=stg[:], in_=d_[:])
                        nc.vector.tensor_copy(t_[:], stg[:])
                # init state: tanh(h0) converted to f32r
                stg0 = swpool.tile([128, HB], F32, tag="stg0")
                nc.vector.memset(stg0[:], TANH_H0)
                nc.vector.tensor_copy(tanhT_state[0][:], stg0[:])
                nc.vector.tensor_copy(tanhT_state[1][:], stg0[:])

                with tc.For_i(0, S, U) as t0:
                    # batched x loads for this body: [128, 2, U*BS]
                    xstg = xpool.tile([128, DTILES, U * BS], F32, tag="xstg")
                    for dt_ in range(DTILES):
                        nc.sync.dma_start(out=xstg[:, dt_, :],
                                          in_=xT_d[dt_, :, ds(t0, U), :])
                    xblk = xpool.tile([128, DTILES, U * BS], F32R, tag="xb")
                    nc.vector.tensor_copy(xblk[:], xstg[:])
                    # h outputs accumulated for this body
                    hTblk = hpool.tile([128, U, KT * BS], F32R, tag="hTblk")

                    cur = tanhT_state
                    for u in range(U):
                        # -- GEMM1: z[b, j] = tanh(h) @ w0.T + x_t @ w_ih --
                        # Output half 0 fully first so its post-chain
                        # overlaps half 1's matmuls.  Within a half, state
                        # half 0 (kt 0-3) first: it is ready before state
                        # half 1 during the previous step's tail.
                        pz = [pp1.tile([BS, 512], F32, tag=f"pz{i}",
                                       name=f"pz{i}") for i in range(2)]
                        for ho in range(2):
                            o0 = ho * 512
                            for kt in range(KT):
                                sh, si = divmod(kt, 4)
                                nc.tensor.matmul(
                                    pz[ho][:],
                                    cur[sh][:, si * BS:(si + 1) * BS],
                                    w0T[:, kt * H + o0: kt * H + o0 + 512],
                                    start=(kt == 0), stop=False)
                            for dt_ in range(DTILES):
                                nc.tensor.matmul(
                                    pz[ho][:],
                                    xblk[:, dt_, u * BS:(u + 1) * BS],
                                    wih[:, dt_ * H + o0: dt_ * H + o0 + 512],
                                    start=False, stop=(dt_ == DTILES - 1))
                        # z-post per half: clip(z+b1)^p, transpose
                        z2T = []
                        for hf in range(2):
                            o0 = hf * 512
                            znt = wpool.tile([BS, 512], F32, tag=f"znt{hf}")
                            nc.vector.tensor_add(znt[:], pz[hf][:],
                                                 b1bc[:, o0:o0 + 512])
                            nc.vector.tensor_scalar(znt[:], znt[:], 0.0, 1.0,
                                                    op0=ALU.max, op1=ALU.min)
                            pzT = pp2.tile([128, HB], F32, tag=f"pzT{hf}")
                            for q in range(4):
                                nc.tensor.transpose(
                                    pzT[:, q * BS:(q + 1) * BS],
                                    znt[0:BS, q * 128:(q + 1) * 128], idn16)
                            zz = wpool.tile([128, HB], F32R, tag=f"z2T{hf}")
                            if poly_power == 2:
                                nc.scalar.activation(zz[:], pzT[:], AF.Square)
                            elif poly_power == 1:
                                nc.scalar.activation(zz[:], pzT[:], AF.Copy)
                            else:
                                zT = wpool.tile([128, HB], F32, tag=f"zTg{hf}")
                                nc.scalar.activation(zT[:], pzT[:], AF.Copy)
                                zp = wpool.tile([128, HB], F32, tag=f"zpg{hf}")
                                if poly_power == 0:
                                    nc.vector.memset(zp[:], 1.0)
                                else:
                                    nc.scalar.activation(zp[:], pzT[:],
                                                         AF.Square)
                                    for _ in range(poly_power - 2):
                                        nc.vector.tensor_mul(zp[:], zp[:],
                                                             zT[:])
                                nc.vector.tensor_copy(zz[:], zp[:])
                            z2T.append(zz)
                        # -- GEMM2: h[b, j'] = z2 @ w1.T --
                        # Contraction half outer: the kh=0 matmuls only
                        # need z2T[0], so they overlap half 1's z-post.
                        ph = [pp1.tile([BS, 512], F32, tag=f"ph{i}",
                                       name=f"ph{i}") for i in range(2)]
                        for kh in range(2):
                            for ho in range(2):
                                o0 = ho * 512
                                for q in range(4):
                                    kt = kh * 4 + q
                                    nc.tensor.matmul(
                                        ph[ho][:],
                                        z2T[kh][:, q * BS:(q + 1) * BS],
                                        w1T[:, kt * H + o0: kt * H + o0 + 512],
                                        start=(kh == 0 and q == 0),
                                        stop=(kh == 1 and q == 3))
                        # h-post per half: +b2, transpose, stage, tanh
                        nxt = []
                        for hf in range(2):
                            o0 = hf * 512
                            hnt = wpool.tile([BS, 512], F32, tag=f"hnt{hf}")
                            nc.vector.tensor_add(hnt[:], ph[hf][:],
                                                 b2bc[:, o0:o0 + 512])
                            phT = pp2.tile([128, HB], F32, tag=f"phT{hf}")
                            for q in range(4):
                                nc.tensor.transpose(
                                    phT[:, q * BS:(q + 1) * BS],
                                    hnt[0:BS, q * 128:(q + 1) * 128], idn16)
                            nc.scalar.activation(
                                hTblk[:, u, hf * HB:(hf + 1) * HB],
                                phT[:], AF.Copy)
                            nx = tanhT_state[hf] if u == U - 1 else \
                                wpool.tile([128, HB], F32R, tag=f"th{hf}")
                            nc.scalar.activation(nx[:], phT[:], AF.Tanh)
                            nxt.append(nx)
                        cur = nxt

                    for kt in range(KT):
                        nc.sync.dma_start(
                            out=hsT_d[kt, :, ds(t0 * BS, U * BS)],
                            in_=hTblk[:, :, kt * BS:(kt + 1) * BS])

            # ---- phase 2: readout ----
            with (
                tc.tile_pool(name="rw", bufs=1) as rwpool,
                tc.tile_pool(name="p2", bufs=1) as p2pool,
                tc.tile_pool(name="p2s", bufs=2) as p2spool,
                tc.tile_pool(name="ps3", bufs=2, space="PSUM") as pp3,
            ):
                hl1T = rwpool.tile([128, KT * H], F32R, tag="hl1T")
                roT = rwpool.tile([128, KT * O], F32R, tag="roT")
                hl1bc = rwpool.tile([128, KT], F32, tag="hl1bc")
                robc = rwpool.tile([128, OT], F32, tag="robc")
                for t_, d_ in ((hl1bc, hl1bc_d), (robc, robc_d)):
                    nc.sync.dma_start(out=t_[:], in_=d_[:])
                with tc.tile_pool(name="stg2", bufs=2) as stg2pool:
                    for t_, d_ in ((hl1T, hl1T_d), (roT, roT_d)):
                        stg = stg2pool.tile(list(t_.shape), F32, tag="stg2")
                        nc.sync.dma_start(out=stg[:], in_=d_[:])
                        nc.vector.tensor_copy(t_[:], stg[:])

                for c in range(NCH):
                    n0 = c * CHUNK
                    hsc = p2pool.tile([128, KT, CHUNK], F32R, tag="hsc")
                    for kt in range(KT):
                        nc.sync.dma_start(out=hsc[:, kt, :],
                                          in_=hsT_d[kt, :, n0:n0 + CHUNK])
                    hidT = p2pool.tile([128, KT, CHUNK], F32R, tag="hidT")
                    for gt in range(KT):
                        phid = pp3.tile([128, CHUNK], F32, tag="phid")
                        for ht in range(KT):
                            nc.tensor.matmul(
                                phid[:],
                                hl1T[:, ht * H + gt * 128:
                                     ht * H + (gt + 1) * 128],
                                hsc[:, ht, :],
                                start=(ht == 0), stop=(ht == KT - 1))
                        nc.scalar.activation(hidT[:, gt, :], phid[:], AF.Relu,
                                             bias=hl1bc[:, gt:gt + 1])
                    for ot in range(OT):
                        po = pp3.tile([128, CHUNK], F32, tag="po")
                        for gt in range(KT):
                            nc.tensor.matmul(
                                po[:],
                                roT[:, gt * O + ot * 128:
                                    gt * O + (ot + 1) * 128],
                                hidT[:, gt, :],
                                start=(gt == 0), stop=(gt == KT - 1))
                        osb = p2spool.tile([128, CHUNK], F32, tag="osb")
                        nc.vector.tensor_scalar_add(osb[:], po[:],
                                                    robc[:, ot:ot + 1])
                        nc.sync.dma_start(out=outT_d[ot, :, n0:n0 + CHUNK],
                                          in_=osb[:])

    nc.compile()
    return nc


def _get_nc(poly_power: int):
    if poly_power not in _BUILD_CACHE:
        _BUILD_CACHE[poly_power] = _build(poly_power)
    return _BUILD_CACHE[poly_power]


def _tile_kmajor(w, kt, width):
    """[K, width] -> [128, kt*width] with contraction tiled on partitions."""
    return np.ascontiguousarray(
        w.reshape(kt, 128, width).transpose(1, 0, 2).reshape(128, kt * width))


def kernel(x, w_ih, w_hh0, w_hh1, b1, b2, hl1_w, hl1_b, ro_w, ro_b,
           poly_power):
    global LAST_RESULTS
    x = np.asarray(x, np.float32)
    w_ih = np.asarray(w_ih, np.float32)
    w_hh0 = np.asarray(w_hh0, np.float32)
    w_hh1 = np.asarray(w_hh1, np.float32)
    b1 = np.asarray(b1, np.float32)
    b2 = np.asarray(b2, np.float32)
    hl1_w = np.asarray(hl1_w, np.float32)
    hl1_b = np.asarray(hl1_b, np.float32)
    ro_w = np.asarray(ro_w, np.float32)
    ro_b = np.asarray(ro_b, np.float32)
    p = int(poly_power)

    nc = _get_nc(p)

    # host layout prep (shared across cores)
    w0T = _tile_kmajor(np.ascontiguousarray(w_hh0.T), KT, H)
    wih = _tile_kmajor(w_ih, DTILES, H)
    w1T = _tile_kmajor(np.ascontiguousarray(w_hh1.T), KT, H)
    hl1T = _tile_kmajor(np.ascontiguousarray(hl1_w.T), KT, H)
    roT = _tile_kmajor(np.ascontiguousarray(ro_w.T), KT, O)
    b1bc = np.ascontiguousarray(np.broadcast_to(b1, (BS, H)))
    b2bc = np.ascontiguousarray(np.broadcast_to(b2, (BS, H)))
    hl1bc = np.ascontiguousarray(hl1_b.reshape(KT, 128).T)
    robc = np.ascontiguousarray(ro_b.reshape(OT, 128).T)

    # x: [S, B, D] -> per-core [DTILES, 128, S, BS]
    xT = np.ascontiguousarray(
        x.transpose(2, 0, 1).reshape(DTILES, 128, S, B))

    shared = dict(w0T=w0T, wih=wih, w1T=w1T, hl1T=hl1T, roT=roT,
                  b1bc=b1bc, b2bc=b2bc, hl1bc=hl1bc, robc=robc)
    in_maps = []
    for i in range(NCORES):
        m = dict(shared)
        m["xT"] = np.ascontiguousarray(xT[:, :, :, i * BS:(i + 1) * BS])
        in_maps.append(m)

    import os as _os
    _trace = _os.environ.get("KERNEL_TRACE", "") == "1"
    _kw = {}
    if _trace:
        _kw = dict(trace=True, tmpdir=_os.environ.get("KERNEL_TRACE_DIR") or None)
    res = run_bass_kernel_spmd(nc, in_maps, list(range(NCORES)), **_kw)
    LAST_RESULTS = res

    out = np.empty((S, B, O), np.float32)
    for i in range(NCORES):
        oT = res.results[i]["outT"]  # [OT, 128, S*BS]
        out[:, i * BS:(i + 1) * BS, :] = (
            oT.reshape(OT, 128, S, BS).transpose(2, 3, 0, 1)
            .reshape(S, BS, O))
    return out


# revision 12
# speedup vs baseline: 1.3985x; 1.0405x over previous
"""Trainium2 Bass kernel for nn_PolyEpisodicRNNModel.

Model (per reference):
    h0 = 0.001;  scan over S steps:
        z = clip(tanh(h) @ w_hh0.T + x_t @ w_ih + b1, 0, 1)
        h = z**poly_power @ w_hh1.T + b2          (hs collects h)
    readout per step: out = relu(hs @ hl1_w.T + hl1_b) @ ro_w.T + ro_b

Strategy:
  - Data-parallel over batch: B=128 -> 16 per core on 8 NeuronCores.
  - The scan is serial; per step the PE must stream both HxH weight
    matrices.  We keep the *weights* as the moving operand (N=512 per
    matmul, float32r -> 1 cycle/row) and the small [128,16] activation
    tiles as the stationary operand (cheap weight loads).
  - State is carried transposed ([H, Bshard], H on partitions) via PE
    transposes of the [16, 1024] z/h tiles each step.
  - hs is staged to DRAM transposed; a static phase-2 readout
    (W-stationary, n=S*16 moving) computes the output per core.
  - All layout transforms (weight transposes, x transpose) happen on the
    host in numpy: they are free w.r.t. HW time.
"""

import sys

for _p in ("/opt/trn_rl_repo",):
    if _p not in sys.path:
        sys.path.insert(0, _p)

import numpy as np

import concourse.bass as bass
import concourse.bacc as bacc
import concourse.mybir as mybir
import concourse.tile as tile
from concourse.bass import ds
from concourse.bass_utils import run_bass_kernel_spmd
from concourse.masks import make_identity

S, B, D, H, O = 512, 128, 256, 1024, 256
NCORES = 8
BS = B // NCORES            # 16 batch rows per core
KT = H // 128               # 8 contraction tiles over H
DTILES = D // 128           # 2 contraction tiles over D
OT = O // 128               # 2 output tiles
U = 32                      # scan steps unrolled per For_i iteration
HB = 4 * (B // NCORES)      # 64: half of the transposed state's columns
N_TOT = S * BS              # 8192 readout columns per core
CHUNK = 512                 # readout moving-dim chunk
NCH = N_TOT // CHUNK        # 16 chunks

F32 = mybir.dt.float32
F32R = mybir.dt.float32r
AF = mybir.ActivationFunctionType
ALU = mybir.AluOpType

TANH_H0 = float(np.tanh(0.001))

_BUILD_CACHE = {}
LAST_RESULTS = None


def _r(ap):
    """View an fp32 AP as float32r for full-speed PE matmuls."""
    return ap.bitcast(F32R)


def _build(poly_power: int):
    nc = bacc.Bacc("TRN2", target_bir_lowering=False, debug=False,
                   num_devices=NCORES)

    # ---- DRAM parameters (host-prepped layouts) ----
    xT_d = nc.dram_tensor("xT", [DTILES, 128, S, BS], F32, kind="ExternalInput")
    w0T_d = nc.dram_tensor("w0T", [128, KT * H], F32, kind="ExternalInput")
    wih_d = nc.dram_tensor("wih", [128, DTILES * H], F32, kind="ExternalInput")
    w1T_d = nc.dram_tensor("w1T", [128, KT * H], F32, kind="ExternalInput")
    hl1T_d = nc.dram_tensor("hl1T", [128, KT * H], F32, kind="ExternalInput")
    roT_d = nc.dram_tensor("roT", [128, KT * O], F32, kind="ExternalInput")
    b1bc_d = nc.dram_tensor("b1bc", [BS, H], F32, kind="ExternalInput")
    b2bc_d = nc.dram_tensor("b2bc", [BS, H], F32, kind="ExternalInput")
    hl1bc_d = nc.dram_tensor("hl1bc", [128, KT], F32, kind="ExternalInput")
    robc_d = nc.dram_tensor("robc", [128, OT], F32, kind="ExternalInput")

    outT_d = nc.dram_tensor("outT", [OT, 128, N_TOT], F32, kind="ExternalOutput")
    hsT_d = nc.dram_tensor("hsT", [KT, 128, N_TOT], F32R)  # internal scratch

    with tile.TileContext(nc) as tc:
        with tc.tile_pool(name="const", bufs=1) as cpool:
            idn = cpool.tile([128, 128], F32, tag="idn")
            make_identity(nc, idn[:])
            idn16 = idn[0:BS, 0:BS]

            # loop-carried state: tanh(h).T in two halves [128, 4*BS] each
            tanhT_state = [cpool.tile([128, HB], F32R, tag=f"tanhT{i}",
                                      name=f"tanhT{i}") for i in range(2)]

            # ---- phase 1: the scan ----
            with (
                tc.tile_pool(name="sw", bufs=1) as swpool,
                tc.tile_pool(name="work", bufs=1) as wpool,
                tc.tile_pool(name="xin", bufs=2) as xpool,
                tc.tile_pool(name="hout", bufs=1) as hpool,
                tc.tile_pool(name="ps1", bufs=1, space="PSUM") as pp1,
                tc.tile_pool(name="ps2", bufs=1, space="PSUM") as pp2,
            ):
                w0T = swpool.tile([128, KT * H], F32R, tag="w0T")
                wih = swpool.tile([128, DTILES * H], F32R, tag="wih")
                w1T = swpool.tile([128, KT * H], F32R, tag="w1T")
                b1bc = swpool.tile([BS, H], F32, tag="b1bc")
                b2bc = swpool.tile([BS, H], F32, tag="b2bc")
                for t_, d_ in ((b1bc, b1bc_d), (b2bc, b2bc_d)):
                    nc.sync.dma_start(out=t_[:], in_=d_[:])
                with tc.tile_pool(name="stg", bufs=2) as stgpool:
                    for t_, d_ in ((w0T, w0T_d), (wih, wih_d), (w1T, w1T_d)):
                        stg = stgpool.tile(list(t_.shape), F32, tag="stg")
                        nc.sync.dma_start(out=stg[:], in_=d_[:])
                        nc.vector.tensor_copy(t_[:], stg[:])
                # init state: tanh(h0) converted to f32r
                stg0 = swpool.tile([128, HB], F32, tag="stg0")
                nc.vector.memset(stg0[:], TANH_H0)
                nc.vector.tensor_copy(tanhT_state[0][:], stg0[:])
                nc.vector.tensor_copy(tanhT_state[1][:], stg0[:])

                with tc.For_i(0, S, U,
                              hint_engines=(mybir.EngineType.PE,)) as t0:
                    # batched x loads for this body: [128, 2, U*BS]
                    xstg = xpool.tile([128, DTILES, U * BS], F32, tag="xstg")
                    for dt_ in range(DTILES):
                        nc.sync.dma_start(out=xstg[:, dt_, :],
                                          in_=xT_d[dt_, :, ds(t0, U), :])
                    xblk = xpool.tile([128, DTILES, U * BS], F32R, tag="xb")
                    nc.vector.tensor_copy(xblk[:], xstg[:])
                    # h outputs accumulated for this body
                    hTblk = hpool.tile([128, U, KT * BS], F32R, tag="hTblk")

                    cur = tanhT_state
                    for u in range(U):
                        # -- GEMM1: z[b, j] = tanh(h) @ w0.T + x_t @ w_ih --
                        # Output half 0 fully first so its post-chain
                        # overlaps half 1's matmuls.  Within a half, state
                        # half 0 (kt 0-3) first: it is ready before state
                        # half 1 during the previous step's tail.
                        pz = [pp1.tile([BS, 512], F32, tag=f"pz{i}",
                                       name=f"pz{i}") for i in range(2)]
                        for ho in range(2):      # early: needs state half 0
                            o0 = ho * 512
                            for kt in range(4):
                                nc.tensor.matmul(
                                    pz[ho][:],
                                    cur[0][:, kt * BS:(kt + 1) * BS],
                                    w0T[:, kt * H + o0: kt * H + o0 + 512],
                                    start=(kt == 0), stop=False)
                        for ho in range(2):      # late: needs state half 1
                            o0 = ho * 512
                            for kt in range(4, KT):
                                nc.tensor.matmul(
                                    pz[ho][:],
                                    cur[1][:, (kt - 4) * BS:(kt - 3) * BS],
                                    w0T[:, kt * H + o0: kt * H + o0 + 512],
                                    start=False, stop=False)
                            for dt_ in range(DTILES):
                                nc.tensor.matmul(
                                    pz[ho][:],
                                    xblk[:, dt_, u * BS:(u + 1) * BS],
                                    wih[:, dt_ * H + o0: dt_ * H + o0 + 512],
                                    start=False, stop=(dt_ == DTILES - 1))
                        # z-post per half: clip(z+b1)^p, transpose
                        z2T = []
                        for hf in range(2):
                            o0 = hf * 512
                            znt = wpool.tile([BS, 512], F32, tag=f"znt{hf}")
                            nc.vector.tensor_add(znt[:], pz[hf][:],
                                                 b1bc[:, o0:o0 + 512])
                            nc.vector.tensor_scalar(znt[:], znt[:], 0.0, 1.0,
                                                    op0=ALU.max, op1=ALU.min)
                            pzT = pp2.tile([128, HB], F32, tag=f"pzT{hf}")
                            for q in range(4):
                                nc.tensor.transpose(
                                    pzT[:, q * BS:(q + 1) * BS],
                                    znt[0:BS, q * 128:(q + 1) * 128], idn16)
                            zz = wpool.tile([128, HB], F32R, tag=f"z2T{hf}")
                            if poly_power == 2:
                                nc.scalar.activation(zz[:], pzT[:], AF.Square)
                            elif poly_power == 1:
                                nc.scalar.activation(zz[:], pzT[:], AF.Copy)
                            else:
                                zT = wpool.tile([128, HB], F32, tag=f"zTg{hf}")
                                nc.scalar.activation(zT[:], pzT[:], AF.Copy)
                                zp = wpool.tile([128, HB], F32, tag=f"zpg{hf}")
                                if poly_power == 0:
                                    nc.vector.memset(zp[:], 1.0)
                                else:
                                    nc.scalar.activation(zp[:], pzT[:],
                                                         AF.Square)
                                    for _ in range(poly_power - 2):
                                        nc.vector.tensor_mul(zp[:], zp[:],
                                                             zT[:])
                                nc.vector.tensor_copy(zz[:], zp[:])
                            z2T.append(zz)
                        # -- GEMM2: h[b, j'] = z2 @ w1.T --
                        # Contraction half outer: the kh=0 matmuls only
                        # need z2T[0], so they overlap half 1's z-post.
                        ph = [pp1.tile([BS, 512], F32, tag=f"ph{i}",
                                       name=f"ph{i}") for i in range(2)]
                        for kh in range(2):
                            for ho in range(2):
                                o0 = ho * 512
                                for q in range(4):
                                    kt = kh * 4 + q
                                    nc.tensor.matmul(
                                        ph[ho][:],
                                        z2T[kh][:, q * BS:(q + 1) * BS],
                                        w1T[:, kt * H + o0: kt * H + o0 + 512],
                                        start=(kh == 0 and q == 0),
                                        stop=(kh == 1 and q == 3))
                        # h-post per half: +b2, transpose, stage, tanh
                        nxt = []
                        for hf in range(2):
                            o0 = hf * 512
                            hnt = wpool.tile([BS, 512], F32, tag=f"hnt{hf}")
                            nc.vector.tensor_add(hnt[:], ph[hf][:],
                                                 b2bc[:, o0:o0 + 512])
                            phT = pp2.tile([128, HB], F32, tag=f"phT{hf}")
                            for q in range(4):
                                nc.tensor.transpose(
                                    phT[:, q * BS:(q + 1) * BS],
                                    hnt[0:BS, q * 128:(q + 1) * 128], idn16)
                            nc.scalar.activation(
                                hTblk[:, u, hf * HB:(hf + 1) * HB],
                                phT[:], AF.Copy)
                            nx = tanhT_state[hf] if u == U - 1 else \
                                wpool.tile([128, HB], F32R, tag=f"th{hf}")
                            nc.scalar.activation(nx[:], phT[:], AF.Tanh)
                            nxt.append(nx)
                        cur = nxt

                    for kt in range(KT):
                        nc.sync.dma_start(
                            out=hsT_d[kt, :, ds(t0 * BS, U * BS)],
                            in_=hTblk[:, :, kt * BS:(kt + 1) * BS])

            # ---- phase 2: readout ----
            with (
                tc.tile_pool(name="rw", bufs=1) as rwpool,
                tc.tile_pool(name="p2", bufs=1) as p2pool,
                tc.tile_pool(name="p2s", bufs=2) as p2spool,
                tc.tile_pool(name="ps3", bufs=2, space="PSUM") as pp3,
            ):
                hl1T = rwpool.tile([128, KT * H], F32R, tag="hl1T")
                roT = rwpool.tile([128, KT * O], F32R, tag="roT")
                hl1bc = rwpool.tile([128, KT], F32, tag="hl1bc")
                robc = rwpool.tile([128, OT], F32, tag="robc")
                for t_, d_ in ((hl1bc, hl1bc_d), (robc, robc_d)):
                    nc.sync.dma_start(out=t_[:], in_=d_[:])
                with tc.tile_pool(name="stg2", bufs=2) as stg2pool:
                    for t_, d_ in ((hl1T, hl1T_d), (roT, roT_d)):
                        stg = stg2pool.tile(list(t_.shape), F32, tag="stg2")
                        nc.sync.dma_start(out=stg[:], in_=d_[:])
                        nc.vector.tensor_copy(t_[:], stg[:])

                for c in range(NCH):
                    n0 = c * CHUNK
                    hsc = p2pool.tile([128, KT, CHUNK], F32R, tag="hsc")
                    for kt in range(KT):
                        nc.sync.dma_start(out=hsc[:, kt, :],
                                          in_=hsT_d[kt, :, n0:n0 + CHUNK])
                    hidT = p2pool.tile([128, KT, CHUNK], F32R, tag="hidT")
                    for gt in range(KT):
                        phid = pp3.tile([128, CHUNK], F32, tag="phid")
                        for ht in range(KT):
                            nc.tensor.matmul(
                                phid[:],
                                hl1T[:, ht * H + gt * 128:
                                     ht * H + (gt + 1) * 128],
                                hsc[:, ht, :],
                                start=(ht == 0), stop=(ht == KT - 1))
                        nc.scalar.activation(hidT[:, gt, :], phid[:], AF.Relu,
                                             bias=hl1bc[:, gt:gt + 1])
                    for ot in range(OT):
                        po = pp3.tile([128, CHUNK], F32, tag="po")
                        for gt in range(KT):
                            nc.tensor.matmul(
                                po[:],
                                roT[:, gt * O + ot * 128:
                                    gt * O + (ot + 1) * 128],
                                hidT[:, gt, :],
                                start=(gt == 0), stop=(gt == KT - 1))
                        osb = p2spool.tile([128, CHUNK], F32, tag="osb")
                        nc.vector.tensor_scalar_add(osb[:], po[:],
                                                    robc[:, ot:ot + 1])
                        nc.sync.dma_start(out=outT_d[ot, :, n0:n0 + CHUNK],
                                          in_=osb[:])

    nc.compile()
    return nc


def _get_nc(poly_power: int):
    if poly_power not in _BUILD_CACHE:
        _BUILD_CACHE[poly_power] = _build(poly_power)
    return _BUILD_CACHE[poly_power]


def _tile_kmajor(w, kt, width):
    """[K, width] -> [128, kt*width] with contraction tiled on partitions."""
    return np.ascontiguousarray(
        w.reshape(kt, 128, width).transpose(1, 0, 2).reshape(128, kt * width))


def kernel(x, w_ih, w_hh0, w_hh1, b1, b2, hl1_w, hl1_b, ro_w, ro_b,
           poly_power):
    global LAST_RESULTS
    x = np.asarray(x, np.float32)
    w_ih = np.asarray(w_ih, np.float32)
    w_hh0 = np.asarray(w_hh0, np.float32)
    w_hh1 = np.asarray(w_hh1, np.float32)
    b1 = np.asarray(b1, np.float32)
    b2 = np.asarray(b2, np.float32)
    hl1_w = np.asarray(hl1_w, np.float32)
    hl1_b = np.asarray(hl1_b, np.float32)
    ro_w = np.asarray(ro_w, np.float32)
    ro_b = np.asarray(ro_b, np.float32)
    p = int(poly_power)

    nc = _get_nc(p)

    # host layout prep (shared across cores)
    w0T = _tile_kmajor(np.ascontiguousarray(w_hh0.T), KT, H)
    wih = _tile_kmajor(w_ih, DTILES, H)
    w1T = _tile_kmajor(np.ascontiguousarray(w_hh1.T), KT, H)
    hl1T = _tile_kmajor(np.ascontiguousarray(hl1_w.T), KT, H)
    roT = _tile_kmajor(np.ascontiguousarray(ro_w.T), KT, O)
    b1bc = np.ascontiguousarray(np.broadcast_to(b1, (BS, H)))
    b2bc = np.ascontiguousarray(np.broadcast_to(b2, (BS, H)))
    hl1bc = np.ascontiguousarray(hl1_b.reshape(KT, 128).T)
    robc = np.ascontiguousarray(ro_b.reshape(OT, 128).T)

    # x: [S, B, D] -> per-core [DTILES, 128, S, BS]
    xT = np.ascontiguousarray(
        x.transpose(2, 0, 1).reshape(DTILES, 128, S, B))

    shared = dict(w0T=w0T, wih=wih, w1T=w1T, hl1T=hl1T, roT=roT,
                  b1bc=b1bc, b2bc=b2bc, hl1bc=hl1bc, robc=robc)
    in_maps = []
    for i in range(NCORES):
        m = dict(shared)
        m["xT"] = np.ascontiguousarray(xT[:, :, :, i * BS:(i + 1) * BS])
        in_maps.append(m)

    import os as _os
    _trace = _os.environ.get("KERNEL_TRACE", "") == "1"
    _kw = {}
    if _trace:
        _kw = dict(trace=True, tmpdir=_os.environ.get("KERNEL_TRACE_DIR") or None)
    res = run_bass_kernel_spmd(nc, in_maps, list(range(NCORES)), **_kw)
    LAST_RESULTS = res

    out = np.empty((S, B, O), np.float32)
    for i in range(NCORES):
        oT = res.results[i]["outT"]  # [OT, 128, S*BS]
        out[:, i * BS:(i + 1) * BS, :] = (
            oT.reshape(OT, 128, S, BS).transpose(2, 3, 0, 1)
            .reshape(S, BS, O))
    return out
